# revision 1
# baseline (speedup 1.0000x reference)
"""DeepSeek-style MLA decode attention (batch=8, 128 heads, cache 512) on 8 NeuronCores.

Sharding: tensor-parallel over heads (16 heads/core).
 - q LoRA path sharded over the rank dim (Wq_down cols / Wq_up rows); partial
   q summed+scattered to head owners with a ReduceScatter.
 - Wkv_down replicated (c_kv computed fully on every core).
 - k_cache passed host-pretransposed as [h, b, d, keys]; v_cache as [h, b, keys, d].
 - o_proj input rows sharded by head; partial outputs ReduceScattered over the
   batch dim (core b returns batch b's final row).

Note: the reference's "new token" softmax is over a length-1 axis (== 1.0), so
k_new/Wk_up are dead and the new-token contribution is simply + v_new.
"""

import numpy as np

import concourse.bass as bass
import concourse.mybir as mybir
import concourse.tile as tile
from concourse import bacc
from concourse import bass_utils
from concourse.masks import make_identity

NC_ = 8                      # cores
B = 8                        # batch
H = 128                      # total heads
HP = H // NC_                # 16 heads per core
D = 128                      # head dim
L = 512                      # cache len
HID = 7168
QL = 1536
QLP = QL // NC_              # 192
KVL = 512
NH = HP * D                  # 2048 per-core head cols
SCALE = 1.0 / float(np.sqrt(D))
F32 = mybir.dt.float32
# float32r: single-pass fp32 matmul mode (1 cycle/row at N>=256 vs 4 for
# two-pass fp32). Slightly reduced multiply precision; flip off if the
# accuracy gate complains.
USE_F32R = True


F32R = mybir.dt.float32r
MMD = F32R if USE_F32R else F32  # dtype for matmul-operand tiles


def _rb(ap):
    """Bitcast a DRAM f32 source AP for DMA into a float32r tile."""
    return ap.bitcast(F32R) if USE_F32R else ap


def build_nc():
    nc = bacc.Bacc(
        "TRN2",
        target_bir_lowering=False,
        debug=False,
        enable_asserts=True,
        num_devices=NC_,
    )
    xt = nc.dram_tensor("xt", [HID, B], F32, kind="ExternalInput").ap()
    w_down = nc.dram_tensor("w_down", [HID, QLP + KVL], F32, kind="ExternalInput").ap()
    wq_up = nc.dram_tensor("wq_up", [QLP, H * D], F32, kind="ExternalInput").ap()
    wv_up = nc.dram_tensor("wv_up", [KVL, NH], F32, kind="ExternalInput").ap()
    kt = nc.dram_tensor("kt", [32, 128, 2048], F32, kind="ExternalInput").ap()
    v = nc.dram_tensor("v", [32, 128, 2048], F32, kind="ExternalInput").ap()
    wo = nc.dram_tensor("wo", [NH, HID], F32, kind="ExternalInput").ap()
    o = nc.dram_tensor("o", [1, HID], F32, kind="ExternalOutput").ap()

    rg = [list(range(NC_))]

    with tile.TileContext(nc) as tc:
        with (
            tc.tile_pool(name="const", bufs=1) as constp,
            tc.tile_pool(name="sbuf", bufs=1) as sb,
            tc.tile_pool(name="stage", bufs=2) as stg,
            tc.tile_pool(name="wdown", bufs=3) as wdp,
            tc.tile_pool(name="wqup", bufs=2) as wqp,
            tc.tile_pool(name="ktp", bufs=3) as ktp,
            tc.tile_pool(name="vp", bufs=3) as vp,
            tc.tile_pool(name="wop", bufs=3) as wop,
            tc.tile_pool(name="psbank", bufs=6, space="PSUM") as psbank,
            tc.tile_pool(name="pstr", bufs=2, space="PSUM") as pstr,
            tc.tile_pool(name="dram", bufs=1, space="DRAM") as dram,
        ):
            ident = constp.tile([128, 128], F32)
            make_identity(nc, ident[:])
            id8 = ident[0:8, 0:8]
            # uint8 one-hot columns for CopyPredicated masks (must be int dtype)
            identu8 = constp.tile([128, 128], mybir.dt.uint8, tag="identu8")
            nc.vector.tensor_copy(identu8[:], ident[:])

            # ---------------- q path: cdown = x @ [Wq_down_c | Wkv_down] ----------------
            xt_sb = constp.tile([128, 56 * B], MMD, tag="xt")
            nc.sync.dma_start(
                out=xt_sb[:].rearrange("p (c b) -> p c b", c=56),
                in_=_rb(xt).rearrange("(c p) b -> p c b", p=128),
            )
            ps_cd0 = psbank.tile([8, 512], F32, tag="bank")
            ps_cd1 = psbank.tile([8, 512], F32, tag="bank")
            for i in range(56):
                wd_t = wdp.tile([128, QLP + KVL], MMD, tag="wd")
                nc.sync.dma_start(out=wd_t[:], in_=_rb(w_down)[i * 128:(i + 1) * 128, :])
                lhs = xt_sb[:, i * B:(i + 1) * B]
                nc.tensor.matmul(
                    ps_cd0[:8, 0:512], (lhs), (wd_t[:, 0:512]),
                    start=(i == 0), stop=(i == 55),
                )
                nc.tensor.matmul(
                    ps_cd1[:8, 0:192], lhs, wd_t[:, 512:704],
                    start=(i == 0), stop=(i == 55),
                )
            cdown = sb.tile([8, QLP + KVL], F32, tag="cdown")
            nc.vector.tensor_copy(cdown[:, 0:512], ps_cd0[:8, 0:512])
            nc.vector.tensor_copy(cdown[:, 512:704], ps_cd1[:8, 0:192])

            # transposes: cqT [192, 8] (2 chunks), ckvT [512dims -> 4 chunks of [128, 8]]
            ps_cqT = pstr.tile([128, 128], F32, tag="tr")
            nc.tensor.transpose(ps_cqT[0:128, 0:8], cdown[:, 0:128], id8)
            nc.tensor.transpose(ps_cqT[0:64, 8:16], cdown[:, 128:192], id8)
            ps_ckvT = pstr.tile([128, 128], F32, tag="tr")
            for j in range(4):
                nc.tensor.transpose(
                    ps_ckvT[0:128, j * 8:(j + 1) * 8],
                    cdown[:, QLP + j * 128:QLP + (j + 1) * 128],
                    id8,
                )
            cqT = sb.tile([128, 16], MMD, tag="cqT")
            nc.vector.tensor_copy(cqT[:, 0:8], ps_cqT[:, 0:8])
            nc.vector.tensor_copy(cqT[0:64, 8:16], ps_cqT[0:64, 8:16])
            ckvT = sb.tile([128, 32], MMD, tag="ckvT")
            nc.vector.tensor_copy(ckvT[:, 0:32], ps_ckvT[:, 0:32])

            # ---------------- q_part = cq @ Wq_up_c  (8, 16384) ----------------
            # The 8 col-groups of 2048 are exactly the per-core head groups g;
            # store each to q_bounce[g] for the ReduceScatter.
            q_bounce = dram.tile([NC_ * B, NH], F32, tag="qb")
            for g in range(NC_):
                wqA = wqp.tile([128, 2048], MMD, tag="wqA")
                nc.sync.dma_start(
                    out=wqA[:], in_=_rb(wq_up)[0:128, g * 2048:(g + 1) * 2048]
                )
                wqB = wqp.tile([64, 2048], MMD, tag="wqB")
                nc.sync.dma_start(
                    out=wqB[:], in_=_rb(wq_up)[128:192, g * 2048:(g + 1) * 2048]
                )
                qstage = stg.tile([8, NH], F32, tag="qstage")
                for j in range(4):
                    ps_q = psbank.tile([8, 512], F32, tag="bank")
                    nc.tensor.matmul(
                        ps_q[:8, :], (cqT[:, 0:8]), (wqA[:, j * 512:(j + 1) * 512]),
                        start=True, stop=False,
                    )
                    nc.tensor.matmul(
                        ps_q[:8, :], (cqT[0:64, 8:16]), (wqB[:, j * 512:(j + 1) * 512]),
                        start=False, stop=True,
                    )
                    nc.vector.tensor_copy(
                        qstage[:, j * 512:(j + 1) * 512], ps_q[:8, :]
                    )
                nc.sync.dma_start(
                    out=q_bounce[g * B:(g + 1) * B, :], in_=qstage[:]
                )
            q_rs = dram.tile([B, NH], F32, tag="qrs")
            nc.gpsimd.collective_compute(
                "ReduceScatter",
                mybir.AluOpType.add,
                replica_groups=rg,
                ins=[q_bounce.opt()],
                outs=[q_rs.opt()],
            )
            qown = sb.tile([8, NH], F32, tag="qown")
            nc.sync.dma_start(out=qown[:], in_=q_rs[:])

            # ---------------- v_new = ckv @ Wv_up_c  (8, 2048) ----------------
            wvup = constp.tile([128, 4 * NH], MMD, tag="wvup")
            nc.sync.dma_start(
                out=wvup[:].rearrange("p (c n) -> p c n", c=4),
                in_=_rb(wv_up).rearrange("(c p) n -> p c n", p=128),
            )
            vnew = sb.tile([8, NH], F32, tag="vnew")
            for j in range(4):
                ps_v = psbank.tile([8, 512], F32, tag="bank")
                for cc in range(4):
                    nc.tensor.matmul(
                        ps_v[:8, :],
                        (ckvT[:, cc * 8:(cc + 1) * 8]),
                        (wvup[:, cc * NH + j * 512:cc * NH + (j + 1) * 512]),
                        start=(cc == 0), stop=(cc == 3),
                    )
                nc.vector.tensor_copy(vnew[:, j * 512:(j + 1) * 512], ps_v[:8, :])

            # qT [128 d, hb] via 16 transposes
            ps_qT = pstr.tile([128, 128], F32, tag="tr")
            for h in range(HP):
                nc.tensor.transpose(
                    ps_qT[0:128, h * 8:(h + 1) * 8],
                    qown[:, h * D:(h + 1) * D],
                    id8,
                )
            qT = sb.tile([128, 128], MMD, tag="qT")
            nc.vector.tensor_copy(qT[:], ps_qT[:])

            # ---------------- phase A: scores over k cache ----------------
            # lhsT = qT (all 128 hb) stationary; rhs = kT tile (moving, N=512).
            # Out row hb of each full-bank product is the valid score row;
            # extract it with a partition-aligned copy.
            scores = sb.tile([128, 512], F32, tag="scores")
            for t in range(32):
                kt_t = ktp.tile([128, 2048], MMD, tag="kt")
                nc.sync.dma_start(out=kt_t[:], in_=_rb(kt)[t])
                for u in range(4):
                    hb = 4 * t + u
                    ps_s = psbank.tile([128, 512], F32, tag="bank")
                    nc.tensor.matmul(
                        ps_s[:],
                        (qT[:]),
                        (kt_t[:, u * 512:(u + 1) * 512]),
                        start=True, stop=True,
                    )
                    # write only row hb (engines can't address partition hb
                    # directly: start partition must be 0/32/64/96)
                    nc.vector.copy_predicated(
                        scores[:],
                        identu8[:, hb:hb + 1].broadcast_to((128, 512)),
                        ps_s[:],
                    )

            probs = sb.tile([128, 512], F32, tag="probs")
            denom = sb.tile([128, 1], F32, tag="denom")
            nc.scalar.activation(
                probs[:], scores[:], mybir.ActivationFunctionType.Exp,
                scale=SCALE, accum_out=denom[:],
            )
            recip = sb.tile([128, 1], F32, tag="recip")
            nc.vector.reciprocal(recip[:], denom[:])
            probsn = sb.tile([128, 512], F32, tag="probsn")
            nc.vector.tensor_scalar_mul(probsn[:], probs[:], recip[:])

            ps_pT = psbank.tile([128, 512], F32, tag="bank")
            for cc in range(4):
                nc.tensor.transpose(
                    ps_pT[:, cc * 128:(cc + 1) * 128],
                    probsn[:, cc * 128:(cc + 1) * 128],
                    ident[:],
                )
            probsT = sb.tile([128, 512], MMD, tag="probsT")
            nc.vector.tensor_copy(probsT[:], ps_pT[:])

            # ---------------- phase B: attn rows = probs @ V ----------------
            # Per group of 4 hb: lhsT = probsT chunk c (all hb), rhs packs the
            # 4 hb's V chunk c side by side; accumulate over c, then extract
            # row 4g+u from column block u.
            attn = sb.tile([128, 128], F32, tag="attn")
            for g in range(32):
                v_t = vp.tile([128, 2048], MMD, tag="v")
                nc.sync.dma_start(out=v_t[:], in_=_rb(v)[g])
                ps_a = psbank.tile([128, 512], F32, tag="bank")
                for cc in range(4):
                    nc.tensor.matmul(
                        ps_a[:],
                        (probsT[:, cc * 128:(cc + 1) * 128]),
                        (v_t[:, cc * 512:(cc + 1) * 512]),
                        start=(cc == 0), stop=(cc == 3),
                    )
                for u in range(4):
                    hb = 4 * g + u
                    nc.vector.copy_predicated(
                        attn[:],
                        identu8[:, hb:hb + 1].broadcast_to((128, 128)),
                        ps_a[:, u * 128:(u + 1) * 128],
                    )

            # attnT = attn^T + v_new^T
            ps_vT = pstr.tile([128, 128], F32, tag="tr")
            for h in range(HP):
                nc.tensor.transpose(
                    ps_vT[0:128, h * 8:(h + 1) * 8],
                    vnew[:, h * D:(h + 1) * D],
                    id8,
                )
            vnewT = sb.tile([128, 128], F32, tag="vnewT")
            nc.vector.tensor_copy(vnewT[:], ps_vT[:])
            ps_aT = pstr.tile([128, 128], F32, tag="tr")
            nc.tensor.transpose(ps_aT[:], attn[:], ident[:])
            attnT = sb.tile([128, 128], MMD, tag="attnT")
            nc.vector.tensor_add(attnT[:], ps_aT[:], vnewT[:])

            # ---------------- phase C: o_part = attn^T @ Wo_c ----------------
            # Rounds of up to 6 n-chunks so the accumulators fit in the bank
            # pool; Wo streams as per-head row blocks (large contiguous runs).
            o_bounce = dram.tile([B, HID], F32, tag="ob")
            for n0, n1 in ((0, 6), (6, 12), (12, 14)):
                nn = n1 - n0
                ps_os = [
                    psbank.tile([8, 512], F32, tag="bank", name=f"ps_o{n0}_{i}")
                    for i in range(nn)
                ]
                for h in range(HP):
                    wo_t = wop.tile([128, 3072], MMD, tag="wo")
                    nc.sync.dma_start(
                        out=wo_t[:, 0:nn * 512],
                        in_=_rb(wo)[h * D:(h + 1) * D, n0 * 512:n1 * 512],
                    )
                    for i in range(nn):
                        nc.tensor.matmul(
                            ps_os[i][:8, :],
                            (attnT[:, h * 8:(h + 1) * 8]),
                            (wo_t[:, i * 512:(i + 1) * 512]),
                            start=(h == 0), stop=(h == HP - 1),
                        )
                for i in range(nn):
                    ostage = stg.tile([8, 512], F32, tag="ostage")
                    nc.vector.tensor_copy(ostage[:], ps_os[i][:8, :])
                    nc.sync.dma_start(
                        out=o_bounce[:, (n0 + i) * 512:(n0 + i + 1) * 512],
                        in_=ostage[:],
                    )

            o_rs = dram.tile([1, HID], F32, tag="ors")
            nc.gpsimd.collective_compute(
                "ReduceScatter",
                mybir.AluOpType.add,
                replica_groups=rg,
                ins=[o_bounce.opt()],
                outs=[o_rs.opt()],
            )
            nc.sync.dma_start(out=o[:], in_=o_rs[:])

    nc.compile()
    return nc


_NC_CACHE = None


def _get_nc():
    global _NC_CACHE
    if _NC_CACHE is None:
        _NC_CACHE = build_nc()
    return _NC_CACHE


def make_in_maps(x, k_cache, v_cache, Wq_down, Wq_up, Wkv_down, Wv_up, Wo):
    x2 = np.ascontiguousarray(np.asarray(x, dtype=np.float32).reshape(B, HID).T)
    in_maps = []
    for c in range(NC_):
        hs = slice(c * HP, (c + 1) * HP)
        w_down_c = np.ascontiguousarray(
            np.concatenate(
                [Wq_down[:, c * QLP:(c + 1) * QLP], Wkv_down], axis=1
            ).astype(np.float32)
        )
        wq_up_c = np.ascontiguousarray(Wq_up[c * QLP:(c + 1) * QLP, :], dtype=np.float32)
        wv_up_c = np.ascontiguousarray(
            Wv_up[:, c * HP * D:(c + 1) * HP * D], dtype=np.float32
        )
        wo_c = np.ascontiguousarray(
            Wo[c * HP * D:(c + 1) * HP * D, :], dtype=np.float32
        )
        # kt tile g holds hb=4g..4g+4 as [128 d, (t, k)]; hb=(h, b) row-major
        kt_c = np.ascontiguousarray(
            np.asarray(k_cache, dtype=np.float32)[:, hs]
            .transpose(1, 0, 3, 2)          # (16, 8, 128, 512) [h, b, d, k]
            .reshape(32, 4, 128, 512)       # [g, t, d, k]
            .transpose(0, 2, 1, 3)          # [g, d, t, k]
            .reshape(32, 128, 2048)
        )
        # v tile g holds hb=4g..4g+4 as [128 k, (c, t, d)]
        v_c = np.ascontiguousarray(
            np.asarray(v_cache, dtype=np.float32)[:, hs]
            .transpose(1, 0, 2, 3)          # (16, 8, 512, 128) [h, b, l, d]
            .reshape(32, 4, 4, 128, 128)    # [g, t, c, k, d]
            .transpose(0, 3, 2, 1, 4)       # [g, k, c, t, d]
            .reshape(32, 128, 2048)
        )
        in_maps.append(
            {
                "xt": x2,
                "w_down": w_down_c,
                "wq_up": wq_up_c,
                "wv_up": wv_up_c,
                "kt": kt_c,
                "v": v_c,
                "wo": wo_c,
            }
        )
    return in_maps


def kernel(x, k_cache, v_cache, Wq_down, Wq_up, Wkv_down, Wk_up, Wv_up, Wo, **_):
    x = np.asarray(x, dtype=np.float32)
    in_maps = make_in_maps(
        x, np.asarray(k_cache), np.asarray(v_cache),
        np.asarray(Wq_down, dtype=np.float32), np.asarray(Wq_up, dtype=np.float32),
        np.asarray(Wkv_down, dtype=np.float32), np.asarray(Wv_up, dtype=np.float32),
        np.asarray(Wo, dtype=np.float32),
    )
    nc = _get_nc()
    res = bass_utils.run_bass_kernel_spmd(nc, in_maps, core_ids=list(range(NC_)))
    out = np.stack([res.results[b]["o"] for b in range(B)], axis=0)  # (8, 1, 7168)
    return np.ascontiguousarray(out, dtype=np.float32)



# revision 2
# speedup vs baseline: 1.5769x; 1.5769x over previous
"""DeepSeek-style MLA decode attention (batch=8, 128 heads, cache 512) on 8 NeuronCores.

Sharding: tensor-parallel over heads (16 heads/core), all streamed tensors
host-cast to bf16 (halves HBM traffic; the kernel is memory-bound).

 - W_down ([Wq_down | Wkv_down], 7168x2048) row-sharded: each core computes a
   partial c = x @ W_down_rows and a tiny (8,2048) f32 AllReduce completes it.
   This replaces the baseline's (64,2048) q ReduceScatter that serialized the
   whole pipeline for ~100us.
 - Wq_up / Wv_up column-sharded by head; q/v_new computed fully on the owner.
 - k_cache passed host-pretransposed as [h, d, (b, keys)]; v_cache as
   [h, keys_in_chunk, (chunk, b, d)].
 - Wo row-sharded by head; partial outputs ReduceScattered over the batch dim
   in two column halves (first RS overlaps second half's compute).

The reference's "new token" softmax is over a length-1 axis (== 1.0), so
k_new/Wk_up are dead and the new-token contribution is simply + v_new.
"""

import ml_dtypes
import numpy as np

import concourse.bass as bass
import concourse.mybir as mybir
import concourse.tile as tile
from concourse import bacc
from concourse import bass_utils
from concourse.masks import make_identity

NC_ = 8                      # cores
B = 8                        # batch
H = 128                      # total heads
HP = H // NC_                # 16 heads per core
D = 128                      # head dim
L = 512                      # cache len
HID = 7168
QL = 1536
KVL = 512
NH = HP * D                  # 2048 per-core head cols
HROWS = HID // NC_           # 896 hidden rows per core for W_down
SCALE = 1.0 / float(np.sqrt(D))
F32 = mybir.dt.float32
BF16 = mybir.dt.bfloat16
NPBF16 = ml_dtypes.bfloat16


def build_nc():
    nc = bacc.Bacc(
        "TRN2",
        target_bir_lowering=False,
        debug=False,
        enable_asserts=True,
        num_devices=NC_,
    )
    xt = nc.dram_tensor("xt", [128, 7 * B], BF16, kind="ExternalInput").ap()
    wd = nc.dram_tensor("wd", [128, 7 * 2048], BF16, kind="ExternalInput").ap()
    wq = nc.dram_tensor("wq", [128, 12 * 2048], BF16, kind="ExternalInput").ap()
    wv = nc.dram_tensor("wv", [128, 4 * 2048], BF16, kind="ExternalInput").ap()
    kt = nc.dram_tensor("kt", [16, 128, 4096], BF16, kind="ExternalInput").ap()
    v = nc.dram_tensor("v", [16, 128, 4096], BF16, kind="ExternalInput").ap()
    wo = nc.dram_tensor("wo", [2, 16, 128, 3584], BF16, kind="ExternalInput").ap()
    o = nc.dram_tensor("o", [1, HID], F32, kind="ExternalOutput").ap()

    rg = [list(range(NC_))]

    with tile.TileContext(nc) as tc:
        with (
            tc.tile_pool(name="const", bufs=1) as constp,
            tc.tile_pool(name="sbuf", bufs=1) as sb,
            tc.tile_pool(name="stage", bufs=2) as stg,
            tc.tile_pool(name="wqp", bufs=2) as wqp,
            tc.tile_pool(name="ktp", bufs=4) as ktp,
            tc.tile_pool(name="vp", bufs=4) as vp,
            tc.tile_pool(name="wop", bufs=3) as wop,
            tc.tile_pool(name="psbank", bufs=6, space="PSUM") as psbank,
            tc.tile_pool(name="pstr", bufs=2, space="PSUM") as pstr,
            tc.tile_pool(name="dram", bufs=1, space="DRAM") as dram,
        ):
            ident = constp.tile([128, 128], F32)
            make_identity(nc, ident[:])
            id8 = ident[0:8, 0:8]
            # uint8 one-hot columns for CopyPredicated masks (must be int dtype)
            identu8 = constp.tile([128, 128], mybir.dt.uint8, tag="identu8")
            nc.vector.tensor_copy(identu8[:], ident[:])

            # ---------------- partial cdown = x_rows @ [Wq_down | Wkv_down]_rows ----
            xt_sb = constp.tile([128, 7 * B], BF16, tag="xt")
            nc.sync.dma_start(out=xt_sb[:], in_=xt)
            wd_sb = constp.tile([128, 7 * 2048], BF16, tag="wd")
            nc.sync.dma_start(out=wd_sb[:], in_=wd)
            wv_sb = constp.tile([128, 4 * 2048], BF16, tag="wv")
            nc.sync.dma_start(out=wv_sb[:], in_=wv)

            ps_cd = [
                psbank.tile([8, 512], F32, tag="bank", name=f"ps_cd{i}")
                for i in range(4)
            ]
            for i in range(7):
                lhs = xt_sb[:, i * B:(i + 1) * B]
                for j in range(4):
                    nc.tensor.matmul(
                        ps_cd[j][:8, :],
                        lhs,
                        wd_sb[:, i * 2048 + j * 512:i * 2048 + (j + 1) * 512],
                        start=(i == 0), stop=(i == 6),
                    )
            cdp_sb = sb.tile([8, 2048], F32, tag="cdp")
            for j in range(4):
                nc.vector.tensor_copy(cdp_sb[:, j * 512:(j + 1) * 512], ps_cd[j][:8, :])

            # tiny AllReduce completes cdown on every core (~64KB, mesh algo)
            cd_p = dram.tile([8, 2048], F32, tag="cdp_d")
            nc.gpsimd.dma_start(out=cd_p[:], in_=cdp_sb[:])
            cd_r = dram.tile([8, 2048], F32, tag="cdr_d")
            nc.gpsimd.collective_compute(
                "AllReduce",
                mybir.AluOpType.add,
                replica_groups=rg,
                ins=[cd_p.opt()],
                outs=[cd_r.opt()],
            )
            cdown = sb.tile([8, 2048], F32, tag="cdown")
            nc.gpsimd.dma_start(out=cdown[:], in_=cd_r[:])

            # transpose cdown -> cdT [128, 128] bf16: cols j*8 hold chunk j of
            # cq (j<12) / ckv (j>=12)
            ps_cdT = pstr.tile([128, 128], F32, tag="tr")
            for j in range(16):
                nc.tensor.transpose(
                    ps_cdT[0:128, j * 8:(j + 1) * 8],
                    cdown[:, j * 128:(j + 1) * 128],
                    id8,
                )
            cdT = sb.tile([128, 128], BF16, tag="cdT")
            nc.vector.tensor_copy(cdT[:], ps_cdT[:])

            # ---------------- q = cq @ Wq_up_c  (8, 2048) ----------------
            ps_q = [
                psbank.tile([8, 512], F32, tag="bank", name=f"ps_q{i}")
                for i in range(4)
            ]
            for jj in range(6):
                wq_t = wqp.tile([128, 4096], BF16, tag="wq")
                nc.sync.dma_start(out=wq_t[:], in_=wq[:, jj * 4096:(jj + 1) * 4096])
                for s in range(2):
                    j = jj * 2 + s
                    lhsT = cdT[:, j * 8:(j + 1) * 8]
                    for i in range(4):
                        nc.tensor.matmul(
                            ps_q[i][:8, :],
                            lhsT,
                            wq_t[:, s * 2048 + i * 512:s * 2048 + (i + 1) * 512],
                            start=(j == 0), stop=(j == 11),
                        )
            qown = sb.tile([8, 2048], F32, tag="qown")
            for i in range(4):
                nc.vector.tensor_copy(qown[:, i * 512:(i + 1) * 512], ps_q[i][:8, :])

            # qT [128 d, hb] via 16 transposes (hb = h*8 + b)
            ps_qT = pstr.tile([128, 128], F32, tag="tr")
            for h in range(HP):
                nc.tensor.transpose(
                    ps_qT[0:128, h * 8:(h + 1) * 8],
                    qown[:, h * D:(h + 1) * D],
                    id8,
                )
            qT = sb.tile([128, 128], BF16, tag="qT")
            nc.vector.tensor_copy(qT[:], ps_qT[:])

            # ---------------- v_new = ckv @ Wv_up_c  (8, 2048) ----------------
            ps_v = [
                psbank.tile([8, 512], F32, tag="bank", name=f"ps_v{i}")
                for i in range(4)
            ]
            for j in range(4):
                lhsT = cdT[:, 96 + j * 8:96 + (j + 1) * 8]
                for i in range(4):
                    nc.tensor.matmul(
                        ps_v[i][:8, :],
                        lhsT,
                        wv_sb[:, j * 2048 + i * 512:j * 2048 + (i + 1) * 512],
                        start=(j == 0), stop=(j == 3),
                    )
            vnew = sb.tile([8, 2048], F32, tag="vnew")
            for i in range(4):
                nc.vector.tensor_copy(vnew[:, i * 512:(i + 1) * 512], ps_v[i][:8, :])
            ps_vT = pstr.tile([128, 128], F32, tag="tr")
            for h in range(HP):
                nc.tensor.transpose(
                    ps_vT[0:128, h * 8:(h + 1) * 8],
                    vnew[:, h * D:(h + 1) * D],
                    id8,
                )
            vnewT = sb.tile([128, 128], F32, tag="vnewT")
            nc.vector.tensor_copy(vnewT[:], ps_vT[:])

            # ---------------- phase A: scores over k cache ----------------
            # kt tile t holds head t's keys for all batches: [128 d, (b, keys)].
            # lhsT = qT (all 128 hb) stationary; out row hb=8t+u of each product
            # is the valid score row; extract with a partition-aligned
            # predicated copy.
            scores = sb.tile([128, 512], F32, tag="scores")
            for t in range(HP):
                kt_t = ktp.tile([128, 4096], BF16, tag="kt")
                nc.sync.dma_start(out=kt_t[:], in_=kt[t])
                for u in range(8):
                    hb = 8 * t + u
                    ps_s = psbank.tile([128, 512], F32, tag="bank")
                    nc.tensor.matmul(
                        ps_s[:],
                        qT[:],
                        kt_t[:, u * 512:(u + 1) * 512],
                        start=True, stop=True,
                    )
                    nc.vector.copy_predicated(
                        scores[:],
                        identu8[:, hb:hb + 1].broadcast_to((128, 512)),
                        ps_s[:],
                    )

            # softmax: unnormalized probs = exp(scale * scores); fold the
            # 1/denom into the attn rows after phase B.
            probs = sb.tile([128, 512], F32, tag="probs")
            denom = sb.tile([128, 1], F32, tag="denom")
            nc.scalar.activation(
                probs[:], scores[:], mybir.ActivationFunctionType.Exp,
                scale=SCALE, accum_out=denom[:],
            )
            recip = sb.tile([128, 1], F32, tag="recip")
            nc.vector.reciprocal(recip[:], denom[:])

            ps_pT = psbank.tile([128, 512], F32, tag="bank")
            for cc in range(4):
                nc.tensor.transpose(
                    ps_pT[:, cc * 128:(cc + 1) * 128],
                    probs[:, cc * 128:(cc + 1) * 128],
                    ident[:],
                )
            probsT = sb.tile([128, 512], BF16, tag="probsT")
            nc.vector.tensor_copy(probsT[:], ps_pT[:])

            # ---------------- phase B: attn rows = probs @ V ----------------
            # v tile t: [128 l-in-chunk, (chunk cc, b, d)]; accumulate over cc,
            # extract row 8t+uu*4+w from column block w.
            attn = sb.tile([128, 128], F32, tag="attn")
            for t in range(HP):
                v_t = vp.tile([128, 4096], BF16, tag="v")
                nc.sync.dma_start(out=v_t[:], in_=v[t])
                for uu in range(2):
                    ps_a = psbank.tile([128, 512], F32, tag="bank")
                    for cc in range(4):
                        nc.tensor.matmul(
                            ps_a[:],
                            probsT[:, cc * 128:(cc + 1) * 128],
                            v_t[:, cc * 1024 + uu * 512:cc * 1024 + (uu + 1) * 512],
                            start=(cc == 0), stop=(cc == 3),
                        )
                    for w in range(4):
                        hb = 8 * t + uu * 4 + w
                        nc.vector.copy_predicated(
                            attn[:],
                            identu8[:, hb:hb + 1].broadcast_to((128, 128)),
                            ps_a[:, w * 128:(w + 1) * 128],
                        )

            # normalize rows, transpose, add v_new^T, cast bf16
            attn_n = sb.tile([128, 128], F32, tag="attn_n")
            nc.vector.tensor_scalar_mul(attn_n[:], attn[:], recip[:])
            ps_aT = pstr.tile([128, 128], F32, tag="tr")
            nc.tensor.transpose(ps_aT[:], attn_n[:], ident[:])
            attnT = sb.tile([128, 128], BF16, tag="attnT")
            nc.vector.tensor_add(attnT[:], ps_aT[:], vnewT[:])

            # ---------------- phase C: o_part = attn^T @ Wo_c ----------------
            # Two column halves of 3584; each half streams Wo per head-chunk
            # (contiguous 896KB DMAs) into 7 persistent accumulators, then
            # ReduceScatters over the batch dim (core b keeps batch b's row).
            for n in range(2):
                ps_os = [
                    psbank.tile([8, 512], F32, tag="bank", name=f"ps_o{n}_{i}")
                    for i in range(6)
                ]
                ps_os.append(pstr.tile([8, 512], F32, tag="tr", name=f"ps_o{n}_6"))
                for h in range(HP):
                    wo_t = wop.tile([128, 3584], BF16, tag="wo")
                    nc.sync.dma_start(out=wo_t[:], in_=wo[n, h])
                    for i in range(7):
                        nc.tensor.matmul(
                            ps_os[i][:8, :],
                            attnT[:, h * 8:(h + 1) * 8],
                            wo_t[:, i * 512:(i + 1) * 512],
                            start=(h == 0), stop=(h == HP - 1),
                        )
                ob = dram.tile([B, 3584], F32, tag=f"ob{n}", name=f"ob{n}")
                for i in range(7):
                    ostage = stg.tile([8, 512], F32, tag="ostage")
                    nc.vector.tensor_copy(ostage[:], ps_os[i][:8, :])
                    nc.gpsimd.dma_start(
                        out=ob[:, i * 512:(i + 1) * 512], in_=ostage[:]
                    )
                ors = dram.tile([1, 3584], F32, tag=f"ors{n}", name=f"ors{n}")
                nc.gpsimd.collective_compute(
                    "ReduceScatter",
                    mybir.AluOpType.add,
                    replica_groups=rg,
                    ins=[ob.opt()],
                    outs=[ors.opt()],
                )
                nc.gpsimd.dma_start(
                    out=o[:, n * 3584:(n + 1) * 3584], in_=ors[:]
                )

    nc.compile()
    return nc


_NC_CACHE = None


def _get_nc():
    global _NC_CACHE
    if _NC_CACHE is None:
        _NC_CACHE = build_nc()
    return _NC_CACHE


def make_in_maps(x, k_cache, v_cache, Wq_down, Wq_up, Wkv_down, Wv_up, Wo):
    x2 = np.asarray(x, dtype=np.float32).reshape(B, HID)
    k_cache = np.asarray(k_cache, dtype=np.float32)
    v_cache = np.asarray(v_cache, dtype=np.float32)
    wd_full = np.concatenate(
        [np.asarray(Wq_down, dtype=np.float32), np.asarray(Wkv_down, dtype=np.float32)],
        axis=1,
    )  # [7168, 2048]
    Wq_up = np.asarray(Wq_up, dtype=np.float32)
    Wv_up = np.asarray(Wv_up, dtype=np.float32)
    Wo = np.asarray(Wo, dtype=np.float32)

    in_maps = []
    for c in range(NC_):
        hs = slice(c * HP, (c + 1) * HP)
        cols = slice(c * NH, (c + 1) * NH)
        rows = slice(c * HROWS, (c + 1) * HROWS)
        xt_c = np.ascontiguousarray(
            x2[:, rows].T.reshape(7, 128, B).transpose(1, 0, 2).reshape(128, 7 * B)
        ).astype(NPBF16)
        wd_c = np.ascontiguousarray(
            wd_full[rows].reshape(7, 128, 2048).transpose(1, 0, 2).reshape(128, 7 * 2048)
        ).astype(NPBF16)
        wq_c = np.ascontiguousarray(
            Wq_up[:, cols].reshape(12, 128, 2048).transpose(1, 0, 2).reshape(128, 12 * 2048)
        ).astype(NPBF16)
        wv_c = np.ascontiguousarray(
            Wv_up[:, cols].reshape(4, 128, 2048).transpose(1, 0, 2).reshape(128, 4 * 2048)
        ).astype(NPBF16)
        # kt tile t = head t: [d, (b, keys)]
        kt_c = np.ascontiguousarray(
            k_cache[:, hs].transpose(1, 3, 0, 2).reshape(16, 128, 4096)
        ).astype(NPBF16)
        # v tile t = head t: [l-in-chunk, (chunk, b, d)]
        v_c = np.ascontiguousarray(
            v_cache[:, hs]                     # (8 b, 16 h, 512 l, 128 d)
            .reshape(B, HP, 4, 128, 128)       # [b, h, cc, l, d]
            .transpose(1, 3, 2, 0, 4)          # [h, l, cc, b, d]
            .reshape(16, 128, 4096)
        ).astype(NPBF16)
        wo_c = np.ascontiguousarray(
            Wo[cols].reshape(16, 128, 2, 3584).transpose(2, 0, 1, 3)
        ).astype(NPBF16)                       # [2, 16, 128, 3584]
        in_maps.append(
            {
                "xt": xt_c,
                "wd": wd_c,
                "wq": wq_c,
                "wv": wv_c,
                "kt": kt_c,
                "v": v_c,
                "wo": wo_c,
            }
        )
    return in_maps


def kernel(x, k_cache, v_cache, Wq_down, Wq_up, Wkv_down, Wk_up, Wv_up, Wo, **_):
    in_maps = make_in_maps(
        x, k_cache, v_cache, Wq_down, Wq_up, Wkv_down, Wv_up, Wo
    )
    nc = _get_nc()
    res = bass_utils.run_bass_kernel_spmd(nc, in_maps, core_ids=list(range(NC_)))
    out = np.stack([res.results[b]["o"] for b in range(B)], axis=0)  # (8, 1, 7168)
    return np.ascontiguousarray(out, dtype=np.float32)


# revision 4
# speedup vs baseline: 1.8547x; 1.1762x over previous
"""DeepSeek-style MLA decode attention (batch=8, 128 heads, cache 512) on 8 NeuronCores.

Sharding: tensor-parallel over heads (16 heads/core), all streamed tensors
host-cast to bf16 (halves HBM traffic; the kernel is memory-bound).

 - W_down ([Wq_down | Wkv_down], 7168x2048) row-sharded: each core computes a
   partial c = x_rows @ W_down_rows and a tiny (8,2048) f32 AllReduce completes
   it (replaces the baseline's big q ReduceScatter that serialized everything).
 - Wq_up / Wv_up column-sharded by head; q/v_new computed fully on the owner.
 - Scores accumulate into a single PSUM bank via a sliding-window masked lhsT:
   Z is [128, 255] all-zero except column 127 = current q vector, so
   lhsT = Z[:, 127-hb : 255-hb] places q at column hb and the matmul writes
   only score row hb. One [128,1] DVE copy per product instead of a full
   [128,512] predicated extraction (which was 92us of DVE and paced phase A).
 - Wo packed chunk-major [2, 7, 128, 16*512]: each 512-col output chunk
   accumulates over 16 head-blocks in one bank, chunks finish progressively,
   and each half's ReduceScatter overlaps the other half's compute.

The reference's "new token" softmax is over a length-1 axis (== 1.0), so
k_new/Wk_up are dead and the new-token contribution is simply + v_new.
"""

import ml_dtypes
import numpy as np

import concourse.bass as bass
import concourse.mybir as mybir
import concourse.tile as tile
from concourse import bacc
from concourse import bass_utils
from concourse.masks import make_identity

NC_ = 8                      # cores
B = 8                        # batch
H = 128                      # total heads
HP = H // NC_                # 16 heads per core
D = 128                      # head dim
L = 512                      # cache len
HID = 7168
QL = 1536
KVL = 512
NH = HP * D                  # 2048 per-core head cols
HROWS = HID // NC_           # 896 hidden rows per core for W_down
SCALE = 1.0 / float(np.sqrt(D))
F32 = mybir.dt.float32
BF16 = mybir.dt.bfloat16
NPBF16 = ml_dtypes.bfloat16


def build_nc():
    nc = bacc.Bacc(
        "TRN2",
        target_bir_lowering=False,
        debug=False,
        enable_asserts=True,
        num_devices=NC_,
    )
    xt = nc.dram_tensor("xt", [128, 7 * B], BF16, kind="ExternalInput").ap()
    wd = nc.dram_tensor("wd", [128, 7 * 2048], BF16, kind="ExternalInput").ap()
    wq = nc.dram_tensor("wq", [128, 12 * 2048], BF16, kind="ExternalInput").ap()
    wv = nc.dram_tensor("wv", [128, 4 * 2048], BF16, kind="ExternalInput").ap()
    kt = nc.dram_tensor("kt", [16, 128, 4096], BF16, kind="ExternalInput").ap()
    v = nc.dram_tensor("v", [16, 128, 4096], BF16, kind="ExternalInput").ap()
    wo = nc.dram_tensor("wo", [2, 7, 128, 8192], BF16, kind="ExternalInput").ap()
    o = nc.dram_tensor("o", [1, HID], F32, kind="ExternalOutput").ap()

    rg = [list(range(NC_))]

    with tile.TileContext(nc) as tc:
        with (
            tc.tile_pool(name="const", bufs=1) as constp,
            tc.tile_pool(name="sbuf", bufs=1) as sb,
            tc.tile_pool(name="wdp", bufs=2) as wdp,
            tc.tile_pool(name="wqp", bufs=2) as wqp,
            tc.tile_pool(name="ktp", bufs=5) as ktp,
            tc.tile_pool(name="vp", bufs=3) as vp,
            tc.tile_pool(name="wop", bufs=3) as wop,
            tc.tile_pool(name="psbank", bufs=6, space="PSUM") as psbank,
            tc.tile_pool(name="pstr", bufs=2, space="PSUM") as pstr,
            tc.tile_pool(name="dram", bufs=1, space="DRAM") as dram,
        ):
            ident = constp.tile([128, 128], F32)
            make_identity(nc, ident[:])
            id8 = ident[0:8, 0:8]
            # uint8 one-hot columns for CopyPredicated masks (must be int dtype)
            identu8 = constp.tile([128, 128], mybir.dt.uint8, tag="identu8")
            nc.vector.tensor_copy(identu8[:], ident[:])
            # sliding-window masked-lhsT buffers for phase A (col 127 = live q)
            z0 = constp.tile([128, 255], BF16, tag="z0")
            nc.gpsimd.memset(z0[:], 0)
            z1 = constp.tile([128, 255], BF16, tag="z1")
            nc.gpsimd.memset(z1[:], 0)

            # ---------------- partial cdown = x_rows @ [Wq_down | Wkv_down]_rows ----
            xt_sb = constp.tile([128, 7 * B], BF16, tag="xt")
            nc.sync.dma_start(out=xt_sb[:], in_=xt)
            wv_sb = constp.tile([128, 4 * 2048], BF16, tag="wv")
            nc.sync.dma_start(out=wv_sb[:], in_=wv)

            ps_cd = [
                psbank.tile([8, 512], F32, tag="bank", name=f"ps_cd{i}")
                for i in range(4)
            ]
            for i in range(7):
                wd_t = wdp.tile([128, 2048], BF16, tag="wd")
                nc.sync.dma_start(out=wd_t[:], in_=wd[:, i * 2048:(i + 1) * 2048])
                lhs = xt_sb[:, i * B:(i + 1) * B]
                for j in range(4):
                    nc.tensor.matmul(
                        ps_cd[j][:8, :],
                        lhs,
                        wd_t[:, j * 512:(j + 1) * 512],
                        start=(i == 0), stop=(i == 6),
                    )
            cdp_sb = sb.tile([8, 2048], F32, tag="cdp")
            for j in range(4):
                nc.vector.tensor_copy(cdp_sb[:, j * 512:(j + 1) * 512], ps_cd[j][:8, :])

            # tiny AllReduce completes cdown on every core (~64KB, mesh algo)
            cd_p = dram.tile([8, 2048], F32, tag="cdp_d")
            nc.gpsimd.dma_start(out=cd_p[:], in_=cdp_sb[:])
            cd_r = dram.tile([8, 2048], F32, tag="cdr_d")
            nc.gpsimd.collective_compute(
                "AllReduce",
                mybir.AluOpType.add,
                replica_groups=rg,
                ins=[cd_p.opt()],
                outs=[cd_r.opt()],
            )
            cdown = sb.tile([8, 2048], F32, tag="cdown")
            nc.gpsimd.dma_start(out=cdown[:], in_=cd_r[:])

            # transpose cdown -> cdT [128, 128] bf16: cols j*8 hold chunk j of
            # cq (j<12) / ckv (j>=12)
            ps_cdT = pstr.tile([128, 128], F32, tag="tr")
            for j in range(16):
                nc.tensor.transpose(
                    ps_cdT[0:128, j * 8:(j + 1) * 8],
                    cdown[:, j * 128:(j + 1) * 128],
                    id8,
                )
            cdT = sb.tile([128, 128], BF16, tag="cdT")
            nc.vector.tensor_copy(cdT[:], ps_cdT[:])

            # ---------------- q = cq @ Wq_up_c  (8, 2048) ----------------
            ps_q = [
                psbank.tile([8, 512], F32, tag="bank", name=f"ps_q{i}")
                for i in range(4)
            ]
            for jj in range(6):
                wq_t = wqp.tile([128, 4096], BF16, tag="wq")
                nc.sync.dma_start(out=wq_t[:], in_=wq[:, jj * 4096:(jj + 1) * 4096])
                for s in range(2):
                    j = jj * 2 + s
                    lhsT = cdT[:, j * 8:(j + 1) * 8]
                    for i in range(4):
                        nc.tensor.matmul(
                            ps_q[i][:8, :],
                            lhsT,
                            wq_t[:, s * 2048 + i * 512:s * 2048 + (i + 1) * 512],
                            start=(j == 0), stop=(j == 11),
                        )
            qown = sb.tile([8, 2048], F32, tag="qown")
            for i in range(4):
                nc.vector.tensor_copy(qown[:, i * 512:(i + 1) * 512], ps_q[i][:8, :])

            # qT [128 d, hb] via 16 transposes (hb = h*8 + b)
            ps_qT = pstr.tile([128, 128], F32, tag="tr")
            for h in range(HP):
                nc.tensor.transpose(
                    ps_qT[0:128, h * 8:(h + 1) * 8],
                    qown[:, h * D:(h + 1) * D],
                    id8,
                )
            qT = sb.tile([128, 128], BF16, tag="qT")
            nc.vector.tensor_copy(qT[:], ps_qT[:])

            # ---------------- v_new = ckv @ Wv_up_c  (8, 2048) ----------------
            ps_v = [
                psbank.tile([8, 512], F32, tag="bank", name=f"ps_v{i}")
                for i in range(4)
            ]
            for j in range(4):
                lhsT = cdT[:, 96 + j * 8:96 + (j + 1) * 8]
                for i in range(4):
                    nc.tensor.matmul(
                        ps_v[i][:8, :],
                        lhsT,
                        wv_sb[:, j * 2048 + i * 512:j * 2048 + (i + 1) * 512],
                        start=(j == 0), stop=(j == 3),
                    )
            vnew = sb.tile([8, 2048], F32, tag="vnew")
            for i in range(4):
                nc.vector.tensor_copy(vnew[:, i * 512:(i + 1) * 512], ps_v[i][:8, :])
            ps_vT = pstr.tile([128, 128], F32, tag="tr")
            for h in range(HP):
                nc.tensor.transpose(
                    ps_vT[0:128, h * 8:(h + 1) * 8],
                    vnew[:, h * D:(h + 1) * D],
                    id8,
                )
            vnewT = sb.tile([128, 128], F32, tag="vnewT")
            nc.vector.tensor_copy(vnewT[:], ps_vT[:])

            # ---------------- phase A: scores over k cache ----------------
            # kt tile t = head t's keys for all batches: [128 d, (b, keys)].
            # All 128 products accumulate into ONE bank: product hb uses
            # lhsT = Z[:, 127-hb:255-hb] (only column hb nonzero = q_hb), so it
            # writes row hb and adds exact zeros elsewhere.
            ps_sc = psbank.tile([128, 512], F32, tag="bank", name="score_bank")
            for t in range(HP):
                kt_t = ktp.tile([128, 4096], BF16, tag="kt")
                nc.sync.dma_start(out=kt_t[:], in_=kt[t])
                for u in range(8):
                    hb = 8 * t + u
                    z = z0 if (hb & 1) == 0 else z1
                    nc.vector.tensor_copy(z[:, 127:128], qT[:, hb:hb + 1])
                    nc.tensor.matmul(
                        ps_sc[:],
                        z[:, 127 - hb:255 - hb],
                        kt_t[:, u * 512:(u + 1) * 512],
                        start=(hb == 0), stop=(hb == 127),
                    )

            # softmax: unnormalized probs = exp(scale * scores) straight off
            # the bank; fold 1/denom into the attn rows after phase B.
            probs = sb.tile([128, 512], F32, tag="probs")
            denom = sb.tile([128, 1], F32, tag="denom")
            nc.scalar.activation(
                probs[:], ps_sc[:], mybir.ActivationFunctionType.Exp,
                scale=SCALE, accum_out=denom[:],
            )
            recip = sb.tile([128, 1], F32, tag="recip")
            nc.vector.reciprocal(recip[:], denom[:])

            ps_pT = psbank.tile([128, 512], F32, tag="bank")
            for cc in range(4):
                nc.tensor.transpose(
                    ps_pT[:, cc * 128:(cc + 1) * 128],
                    probs[:, cc * 128:(cc + 1) * 128],
                    ident[:],
                )
            probsT = sb.tile([128, 512], BF16, tag="probsT")
            nc.vector.tensor_copy(probsT[:], ps_pT[:])

            # ---------------- phase B: attn rows = probs @ V ----------------
            # v tile t: [128 l-in-chunk, (chunk cc, b, d)]; accumulate over cc,
            # extract row 8t+uu*4+w from column block w.
            attn = sb.tile([128, 128], F32, tag="attn")
            for t in range(HP):
                v_t = vp.tile([128, 4096], BF16, tag="v")
                nc.sync.dma_start(out=v_t[:], in_=v[t])
                for uu in range(2):
                    ps_a = psbank.tile([128, 512], F32, tag="bank")
                    for cc in range(4):
                        nc.tensor.matmul(
                            ps_a[:],
                            probsT[:, cc * 128:(cc + 1) * 128],
                            v_t[:, cc * 1024 + uu * 512:cc * 1024 + (uu + 1) * 512],
                            start=(cc == 0), stop=(cc == 3),
                        )
                    for w in range(4):
                        hb = 8 * t + uu * 4 + w
                        nc.vector.copy_predicated(
                            attn[:],
                            identu8[:, hb:hb + 1].broadcast_to((128, 128)),
                            ps_a[:, w * 128:(w + 1) * 128],
                        )

            # normalize rows, transpose, add v_new^T, cast bf16
            attn_n = sb.tile([128, 128], F32, tag="attn_n")
            nc.vector.tensor_scalar_mul(attn_n[:], attn[:], recip[:])
            ps_aT = pstr.tile([128, 128], F32, tag="tr")
            nc.tensor.transpose(ps_aT[:], attn_n[:], ident[:])
            attnT = sb.tile([128, 128], BF16, tag="attnT")
            nc.vector.tensor_add(attnT[:], ps_aT[:], vnewT[:])

            # ---------------- phase C: o_part = attn^T @ Wo_c ----------------
            # Chunk-major: per (half n, chunk i) one 2MB DMA [128, 16*512] and
            # one accumulating bank over the 16 head-blocks. Chunks complete
            # progressively; each half stores once and ReduceScatters (over the
            # batch dim: core b keeps batch b's row) while the next half runs.
            for n in range(2):
                obuf = sb.tile([8, 3584], F32, tag="obuf", name=f"obuf{n}")
                for i in range(7):
                    wo_t = wop.tile([128, 8192], BF16, tag="wo")
                    nc.sync.dma_start(out=wo_t[:], in_=wo[n, i])
                    ps_o = psbank.tile([8, 512], F32, tag="bank")
                    for h in range(HP):
                        nc.tensor.matmul(
                            ps_o[:8, :],
                            attnT[:, h * 8:(h + 1) * 8],
                            wo_t[:, h * 512:(h + 1) * 512],
                            start=(h == 0), stop=(h == HP - 1),
                        )
                    nc.vector.tensor_copy(obuf[:, i * 512:(i + 1) * 512], ps_o[:8, :])
                ob = dram.tile([B, 3584], F32, tag=f"ob{n}", name=f"ob{n}")
                nc.gpsimd.dma_start(out=ob[:], in_=obuf[:])
                ors = dram.tile([1, 3584], F32, tag=f"ors{n}", name=f"ors{n}")
                nc.gpsimd.collective_compute(
                    "ReduceScatter",
                    mybir.AluOpType.add,
                    replica_groups=rg,
                    ins=[ob.opt()],
                    outs=[ors.opt()],
                )
                nc.gpsimd.dma_start(
                    out=o[:, n * 3584:(n + 1) * 3584], in_=ors[:]
                )

    nc.compile()
    return nc


_NC_CACHE = None


def _get_nc():
    global _NC_CACHE
    if _NC_CACHE is None:
        _NC_CACHE = build_nc()
    return _NC_CACHE


def make_in_maps(x, k_cache, v_cache, Wq_down, Wq_up, Wkv_down, Wv_up, Wo):
    x2 = np.asarray(x, dtype=np.float32).reshape(B, HID)
    k_cache = np.asarray(k_cache, dtype=np.float32)
    v_cache = np.asarray(v_cache, dtype=np.float32)
    wd_full = np.concatenate(
        [np.asarray(Wq_down, dtype=np.float32), np.asarray(Wkv_down, dtype=np.float32)],
        axis=1,
    )  # [7168, 2048]
    Wq_up = np.asarray(Wq_up, dtype=np.float32)
    Wv_up = np.asarray(Wv_up, dtype=np.float32)
    Wo = np.asarray(Wo, dtype=np.float32)

    in_maps = []
    for c in range(NC_):
        hs = slice(c * HP, (c + 1) * HP)
        cols = slice(c * NH, (c + 1) * NH)
        rows = slice(c * HROWS, (c + 1) * HROWS)
        xt_c = np.ascontiguousarray(
            x2[:, rows].T.reshape(7, 128, B).transpose(1, 0, 2).reshape(128, 7 * B)
        ).astype(NPBF16)
        wd_c = np.ascontiguousarray(
            wd_full[rows].reshape(7, 128, 2048).transpose(1, 0, 2).reshape(128, 7 * 2048)
        ).astype(NPBF16)
        wq_c = np.ascontiguousarray(
            Wq_up[:, cols].reshape(12, 128, 2048).transpose(1, 0, 2).reshape(128, 12 * 2048)
        ).astype(NPBF16)
        wv_c = np.ascontiguousarray(
            Wv_up[:, cols].reshape(4, 128, 2048).transpose(1, 0, 2).reshape(128, 4 * 2048)
        ).astype(NPBF16)
        # kt tile t = head t: [d, (b, keys)]
        kt_c = np.ascontiguousarray(
            k_cache[:, hs].transpose(1, 3, 0, 2).reshape(16, 128, 4096)
        ).astype(NPBF16)
        # v tile t = head t: [l-in-chunk, (chunk, b, d)]
        v_c = np.ascontiguousarray(
            v_cache[:, hs]                     # (8 b, 16 h, 512 l, 128 d)
            .reshape(B, HP, 4, 128, 128)       # [b, h, cc, l, d]
            .transpose(1, 3, 2, 0, 4)          # [h, l, cc, b, d]
            .reshape(16, 128, 4096)
        ).astype(NPBF16)
        # wo chunk-major: [(n,i) 14, d 128, (h, 512)] -> [2, 7, 128, 8192]
        wo_c = np.ascontiguousarray(
            Wo[cols].reshape(16, 128, 14, 512).transpose(2, 1, 0, 3)
            .reshape(14, 128, 8192).reshape(2, 7, 128, 8192)
        ).astype(NPBF16)
        in_maps.append(
            {
                "xt": xt_c,
                "wd": wd_c,
                "wq": wq_c,
                "wv": wv_c,
                "kt": kt_c,
                "v": v_c,
                "wo": wo_c,
            }
        )
    return in_maps


def kernel(x, k_cache, v_cache, Wq_down, Wq_up, Wkv_down, Wk_up, Wv_up, Wo, **_):
    in_maps = make_in_maps(
        x, k_cache, v_cache, Wq_down, Wq_up, Wkv_down, Wv_up, Wo
    )
    nc = _get_nc()
    res = bass_utils.run_bass_kernel_spmd(nc, in_maps, core_ids=list(range(NC_)))
    out = np.stack([res.results[b]["o"] for b in range(B)], axis=0)  # (8, 1, 7168)
    return np.ascontiguousarray(out, dtype=np.float32)


# revision 5
# speedup vs baseline: 1.9282x; 1.0397x over previous
"""DeepSeek-style MLA decode attention (batch=8, 128 heads, cache 512) on 8 NeuronCores.

Sharding: tensor-parallel over heads (16 heads/core), all streamed tensors
host-cast to bf16 (halves HBM traffic; the kernel is memory-bound).

 - W_down ([Wq_down | Wkv_down], 7168x2048) row-sharded: each core computes a
   partial c = x_rows @ W_down_rows and a tiny (8,2048) f32 AllReduce completes
   it (replaces the baseline's big q ReduceScatter that serialized everything).
 - Wq_up / Wv_up column-sharded by head; q/v_new computed fully on the owner.
 - Scores accumulate into a single PSUM bank with no per-product DVE work:
   Zbig [128, 16384] is all-zero except columns 127 + 128*hb which hold q_hb
   (built with ONE strided DVE copy). The lhsT window
   Zbig[:, 127*hb+127 : 127*hb+255] then contains exactly one live column at
   position hb, so matmul hb writes score row hb and exact zeros elsewhere.
 - Wo packed chunk-major [2, 7, 128, 16*512]: each 512-col output chunk
   accumulates over 16 head-blocks in one bank, chunks finish progressively,
   each half ReduceScatters (bf16) while the other half computes.

The reference's "new token" softmax is over a length-1 axis (== 1.0), so
k_new/Wk_up are dead and the new-token contribution is simply + v_new.
"""

import ml_dtypes
import numpy as np

import concourse.bass as bass
import concourse.mybir as mybir
import concourse.tile as tile
from concourse import bacc
from concourse import bass_utils
from concourse.masks import make_identity

NC_ = 8                      # cores
B = 8                        # batch
H = 128                      # total heads
HP = H // NC_                # 16 heads per core
D = 128                      # head dim
L = 512                      # cache len
HID = 7168
QL = 1536
KVL = 512
NH = HP * D                  # 2048 per-core head cols
HROWS = HID // NC_           # 896 hidden rows per core for W_down
SCALE = 1.0 / float(np.sqrt(D))
F32 = mybir.dt.float32
BF16 = mybir.dt.bfloat16
NPBF16 = ml_dtypes.bfloat16


def build_nc():
    nc = bacc.Bacc(
        "TRN2",
        target_bir_lowering=False,
        debug=False,
        enable_asserts=True,
        num_devices=NC_,
    )
    xt = nc.dram_tensor("xt", [128, 7 * B], BF16, kind="ExternalInput").ap()
    wd = nc.dram_tensor("wd", [128, 7 * 2048], BF16, kind="ExternalInput").ap()
    wq = nc.dram_tensor("wq", [128, 12 * 2048], BF16, kind="ExternalInput").ap()
    wv = nc.dram_tensor("wv", [128, 4 * 2048], BF16, kind="ExternalInput").ap()
    kt = nc.dram_tensor("kt", [16, 128, 4096], BF16, kind="ExternalInput").ap()
    v = nc.dram_tensor("v", [16, 128, 4096], BF16, kind="ExternalInput").ap()
    wo = nc.dram_tensor("wo", [2, 7, 128, 8192], BF16, kind="ExternalInput").ap()
    o = nc.dram_tensor("o", [1, HID], F32, kind="ExternalOutput").ap()

    rg = [list(range(NC_))]

    with tile.TileContext(nc) as tc:
        with (
            tc.tile_pool(name="const", bufs=1) as constp,
            tc.tile_pool(name="sbuf", bufs=1) as sb,
            tc.tile_pool(name="wdp", bufs=2) as wdp,
            tc.tile_pool(name="wqp", bufs=2) as wqp,
            tc.tile_pool(name="ktp", bufs=4) as ktp,
            tc.tile_pool(name="vp", bufs=3) as vp,
            tc.tile_pool(name="wop", bufs=2) as wop,
            tc.tile_pool(name="psbank", bufs=6, space="PSUM") as psbank,
            tc.tile_pool(name="pstr", bufs=2, space="PSUM") as pstr,
            tc.tile_pool(name="dram", bufs=1, space="DRAM") as dram,
        ):
            ident = constp.tile([128, 128], F32)
            make_identity(nc, ident[:])
            id8 = ident[0:8, 0:8]
            # uint8 one-hot columns for CopyPredicated masks (must be int dtype)
            identu8 = constp.tile([128, 128], mybir.dt.uint8, tag="identu8")
            nc.vector.tensor_copy(identu8[:], ident[:])
            # sparse masked-lhsT buffer for phase A: live cols at 127 + 128*hb
            zbig = constp.tile([128, 16384], BF16, tag="zbig")
            nc.vector.memset(zbig[:], 0)

            # ---------------- partial cdown = x_rows @ [Wq_down | Wkv_down]_rows ----
            xt_sb = constp.tile([128, 7 * B], BF16, tag="xt")
            nc.sync.dma_start(out=xt_sb[:], in_=xt)
            wv_sb = constp.tile([128, 4 * 2048], BF16, tag="wv")
            nc.sync.dma_start(out=wv_sb[:], in_=wv)

            ps_cd = [
                psbank.tile([8, 512], F32, tag="bank", name=f"ps_cd{i}")
                for i in range(4)
            ]
            for i in range(7):
                wd_t = wdp.tile([128, 2048], BF16, tag="wd")
                nc.sync.dma_start(out=wd_t[:], in_=wd[:, i * 2048:(i + 1) * 2048])
                lhs = xt_sb[:, i * B:(i + 1) * B]
                for j in range(4):
                    nc.tensor.matmul(
                        ps_cd[j][:8, :],
                        lhs,
                        wd_t[:, j * 512:(j + 1) * 512],
                        start=(i == 0), stop=(i == 6),
                    )
            cdp_sb = sb.tile([8, 2048], F32, tag="low8", name="cdp_sb")
            for j in range(4):
                nc.vector.tensor_copy(cdp_sb[:, j * 512:(j + 1) * 512], ps_cd[j][:8, :])

            # tiny AllReduce completes cdown on every core (~64KB, mesh algo)
            cd_p = dram.tile([8, 2048], F32, tag="cdp_d")
            nc.gpsimd.dma_start(out=cd_p[:], in_=cdp_sb[:])
            cd_r = dram.tile([8, 2048], F32, tag="cdr_d")
            nc.gpsimd.collective_compute(
                "AllReduce",
                mybir.AluOpType.add,
                replica_groups=rg,
                ins=[cd_p.opt()],
                outs=[cd_r.opt()],
            )
            cdown = sb.tile([8, 2048], F32, tag="low8", name="cdown")
            nc.gpsimd.dma_start(out=cdown[:], in_=cd_r[:])

            # transpose cdown -> cdT [128, 128] bf16: cols j*8 hold chunk j of
            # cq (j<12) / ckv (j>=12)
            ps_cdT = pstr.tile([128, 128], F32, tag="tr")
            for j in range(16):
                nc.tensor.transpose(
                    ps_cdT[0:128, j * 8:(j + 1) * 8],
                    cdown[:, j * 128:(j + 1) * 128],
                    id8,
                )
            cdT = sb.tile([128, 128], BF16, tag="cdT")
            nc.vector.tensor_copy(cdT[:], ps_cdT[:])

            # ---------------- q = cq @ Wq_up_c  (8, 2048) ----------------
            ps_q = [
                psbank.tile([8, 512], F32, tag="bank", name=f"ps_q{i}")
                for i in range(4)
            ]
            for jj in range(6):
                wq_t = wqp.tile([128, 4096], BF16, tag="wq")
                nc.sync.dma_start(out=wq_t[:], in_=wq[:, jj * 4096:(jj + 1) * 4096])
                for s in range(2):
                    j = jj * 2 + s
                    lhsT = cdT[:, j * 8:(j + 1) * 8]
                    for i in range(4):
                        nc.tensor.matmul(
                            ps_q[i][:8, :],
                            lhsT,
                            wq_t[:, s * 2048 + i * 512:s * 2048 + (i + 1) * 512],
                            start=(j == 0), stop=(j == 11),
                        )
            qown = sb.tile([8, 2048], F32, tag="low8", name="qown")
            for i in range(4):
                nc.vector.tensor_copy(qown[:, i * 512:(i + 1) * 512], ps_q[i][:8, :])

            # qT [128 d, hb] via 16 transposes (hb = h*8 + b)
            ps_qT = pstr.tile([128, 128], F32, tag="tr")
            for h in range(HP):
                nc.tensor.transpose(
                    ps_qT[0:128, h * 8:(h + 1) * 8],
                    qown[:, h * D:(h + 1) * D],
                    id8,
                )
            qT = sb.tile([128, 128], BF16, tag="qT")
            nc.vector.tensor_copy(qT[:], ps_qT[:])
            # scatter qT columns into zbig live columns (one strided copy)
            zview = zbig[:].rearrange("p (n s) -> p n s", s=128)
            nc.vector.tensor_copy(
                zview[:, :, 127:128],
                qT[:].rearrange("p (n o) -> p n o", o=1),
            )

            # ---------------- v_new = ckv @ Wv_up_c  (8, 2048) ----------------
            ps_v = [
                psbank.tile([8, 512], F32, tag="bank", name=f"ps_v{i}")
                for i in range(4)
            ]
            for j in range(4):
                lhsT = cdT[:, 96 + j * 8:96 + (j + 1) * 8]
                for i in range(4):
                    nc.tensor.matmul(
                        ps_v[i][:8, :],
                        lhsT,
                        wv_sb[:, j * 2048 + i * 512:j * 2048 + (i + 1) * 512],
                        start=(j == 0), stop=(j == 3),
                    )
            vnew = sb.tile([8, 2048], F32, tag="low8", name="vnew")
            for i in range(4):
                nc.vector.tensor_copy(vnew[:, i * 512:(i + 1) * 512], ps_v[i][:8, :])
            ps_vT = pstr.tile([128, 128], F32, tag="tr")
            for h in range(HP):
                nc.tensor.transpose(
                    ps_vT[0:128, h * 8:(h + 1) * 8],
                    vnew[:, h * D:(h + 1) * D],
                    id8,
                )
            vnewT = sb.tile([128, 128], F32, tag="vnewT")
            nc.vector.tensor_copy(vnewT[:], ps_vT[:])

            # ---------------- phase A: scores over k cache ----------------
            # kt tile t = head t's keys for all batches: [128 d, (b, keys)].
            # All 128 products accumulate into ONE bank; product hb's lhsT
            # window holds q_hb at column hb and zeros elsewhere, so it writes
            # row hb and adds exact zeros to every other row. Pure PE phase.
            ps_sc = psbank.tile([128, 512], F32, tag="bank", name="score_bank")
            for t in range(HP):
                kt_t = ktp.tile([128, 4096], BF16, tag="kt")
                nc.sync.dma_start(out=kt_t[:], in_=kt[t])
                for u in range(8):
                    hb = 8 * t + u
                    nc.tensor.matmul(
                        ps_sc[:],
                        zbig[:, 127 * hb + 127:127 * hb + 255],
                        kt_t[:, u * 512:(u + 1) * 512],
                        start=(hb == 0), stop=(hb == 127),
                    )

            # softmax: unnormalized probs = exp(scale * scores) straight off
            # the bank; fold 1/denom into the attn rows after phase B.
            probs = sb.tile([128, 512], F32, tag="probs")
            denom = sb.tile([128, 1], F32, tag="denom")
            nc.scalar.activation(
                probs[:], ps_sc[:], mybir.ActivationFunctionType.Exp,
                scale=SCALE, accum_out=denom[:],
            )
            recip = sb.tile([128, 1], F32, tag="recip")
            nc.vector.reciprocal(recip[:], denom[:])

            ps_pT = psbank.tile([128, 512], F32, tag="bank")
            for cc in range(4):
                nc.tensor.transpose(
                    ps_pT[:, cc * 128:(cc + 1) * 128],
                    probs[:, cc * 128:(cc + 1) * 128],
                    ident[:],
                )
            probsT = sb.tile([128, 512], BF16, tag="probsT")
            nc.vector.tensor_copy(probsT[:], ps_pT[:])

            # ---------------- phase B: attn rows = probs @ V ----------------
            # v tile t: [128 l-in-chunk, (chunk cc, b, d)]; accumulate over cc,
            # extract row 8t+uu*4+w from column block w.
            attn = sb.tile([128, 128], F32, tag="attn")
            for t in range(HP):
                v_t = vp.tile([128, 4096], BF16, tag="v")
                nc.sync.dma_start(out=v_t[:], in_=v[t])
                for uu in range(2):
                    ps_a = psbank.tile([128, 512], F32, tag="bank")
                    for cc in range(4):
                        nc.tensor.matmul(
                            ps_a[:],
                            probsT[:, cc * 128:(cc + 1) * 128],
                            v_t[:, cc * 1024 + uu * 512:cc * 1024 + (uu + 1) * 512],
                            start=(cc == 0), stop=(cc == 3),
                        )
                    for w in range(4):
                        hb = 8 * t + uu * 4 + w
                        nc.vector.copy_predicated(
                            attn[:],
                            identu8[:, hb:hb + 1].broadcast_to((128, 128)),
                            ps_a[:, w * 128:(w + 1) * 128],
                        )

            # normalize rows, transpose, add v_new^T, cast bf16
            attn_n = sb.tile([128, 128], F32, tag="attn_n")
            nc.vector.tensor_scalar_mul(attn_n[:], attn[:], recip[:])
            ps_aT = pstr.tile([128, 128], F32, tag="tr")
            nc.tensor.transpose(ps_aT[:], attn_n[:], ident[:])
            attnT = sb.tile([128, 128], BF16, tag="attnT")
            nc.vector.tensor_add(attnT[:], ps_aT[:], vnewT[:])

            # ---------------- phase C: o_part = attn^T @ Wo_c ----------------
            # Chunk-major: per (half n, chunk i) one 2MB DMA [128, 16*512] and
            # one accumulating bank over the 16 head-blocks. Chunks complete
            # progressively; each half stores once (HWDGE, so it is not queued
            # behind the previous collective's wait) and ReduceScatters in bf16
            # over the batch dim (core b keeps batch b's row).
            for n in range(2):
                obuf = sb.tile([8, 3584], BF16, tag="obuf", name=f"obuf{n}")
                for i in range(7):
                    wo_t = wop.tile([128, 8192], BF16, tag="wo")
                    nc.sync.dma_start(out=wo_t[:], in_=wo[n, i])
                    ps_o = psbank.tile([8, 512], F32, tag="bank")
                    for h in range(HP):
                        nc.tensor.matmul(
                            ps_o[:8, :],
                            attnT[:, h * 8:(h + 1) * 8],
                            wo_t[:, h * 512:(h + 1) * 512],
                            start=(h == 0), stop=(h == HP - 1),
                        )
                    nc.vector.tensor_copy(obuf[:, i * 512:(i + 1) * 512], ps_o[:8, :])
                ob = dram.tile([B, 3584], BF16, tag=f"ob{n}", name=f"ob{n}")
                nc.sync.dma_start(out=ob[:], in_=obuf[:])
                ors = dram.tile([1, 3584], BF16, tag=f"ors{n}", name=f"ors{n}")
                nc.gpsimd.collective_compute(
                    "ReduceScatter",
                    mybir.AluOpType.add,
                    replica_groups=rg,
                    ins=[ob.opt()],
                    outs=[ors.opt()],
                )
                # SWDGE dma casts bf16 -> f32 on the final store
                nc.gpsimd.dma_start(
                    out=o[:, n * 3584:(n + 1) * 3584], in_=ors[:]
                )

    nc.compile()
    return nc


_NC_CACHE = None


def _get_nc():
    global _NC_CACHE
    if _NC_CACHE is None:
        _NC_CACHE = build_nc()
    return _NC_CACHE


def make_in_maps(x, k_cache, v_cache, Wq_down, Wq_up, Wkv_down, Wv_up, Wo):
    x2 = np.asarray(x, dtype=np.float32).reshape(B, HID)
    k_cache = np.asarray(k_cache, dtype=np.float32)
    v_cache = np.asarray(v_cache, dtype=np.float32)
    wd_full = np.concatenate(
        [np.asarray(Wq_down, dtype=np.float32), np.asarray(Wkv_down, dtype=np.float32)],
        axis=1,
    )  # [7168, 2048]
    Wq_up = np.asarray(Wq_up, dtype=np.float32)
    Wv_up = np.asarray(Wv_up, dtype=np.float32)
    Wo = np.asarray(Wo, dtype=np.float32)

    in_maps = []
    for c in range(NC_):
        hs = slice(c * HP, (c + 1) * HP)
        cols = slice(c * NH, (c + 1) * NH)
        rows = slice(c * HROWS, (c + 1) * HROWS)
        xt_c = np.ascontiguousarray(
            x2[:, rows].T.reshape(7, 128, B).transpose(1, 0, 2).reshape(128, 7 * B)
        ).astype(NPBF16)
        wd_c = np.ascontiguousarray(
            wd_full[rows].reshape(7, 128, 2048).transpose(1, 0, 2).reshape(128, 7 * 2048)
        ).astype(NPBF16)
        wq_c = np.ascontiguousarray(
            Wq_up[:, cols].reshape(12, 128, 2048).transpose(1, 0, 2).reshape(128, 12 * 2048)
        ).astype(NPBF16)
        wv_c = np.ascontiguousarray(
            Wv_up[:, cols].reshape(4, 128, 2048).transpose(1, 0, 2).reshape(128, 4 * 2048)
        ).astype(NPBF16)
        # kt tile t = head t: [d, (b, keys)]
        kt_c = np.ascontiguousarray(
            k_cache[:, hs].transpose(1, 3, 0, 2).reshape(16, 128, 4096)
        ).astype(NPBF16)
        # v tile t = head t: [l-in-chunk, (chunk, b, d)]
        v_c = np.ascontiguousarray(
            v_cache[:, hs]                     # (8 b, 16 h, 512 l, 128 d)
            .reshape(B, HP, 4, 128, 128)       # [b, h, cc, l, d]
            .transpose(1, 3, 2, 0, 4)          # [h, l, cc, b, d]
            .reshape(16, 128, 4096)
        ).astype(NPBF16)
        # wo chunk-major: [(n,i) 14, d 128, (h, 512)] -> [2, 7, 128, 8192]
        wo_c = np.ascontiguousarray(
            Wo[cols].reshape(16, 128, 14, 512).transpose(2, 1, 0, 3)
            .reshape(14, 128, 8192).reshape(2, 7, 128, 8192)
        ).astype(NPBF16)
        in_maps.append(
            {
                "xt": xt_c,
                "wd": wd_c,
                "wq": wq_c,
                "wv": wv_c,
                "kt": kt_c,
                "v": v_c,
                "wo": wo_c,
            }
        )
    return in_maps


def kernel(x, k_cache, v_cache, Wq_down, Wq_up, Wkv_down, Wk_up, Wv_up, Wo, **_):
    in_maps = make_in_maps(
        x, k_cache, v_cache, Wq_down, Wq_up, Wkv_down, Wv_up, Wo
    )
    nc = _get_nc()
    res = bass_utils.run_bass_kernel_spmd(nc, in_maps, core_ids=list(range(NC_)))
    out = np.stack([res.results[b]["o"] for b in range(B)], axis=0)  # (8, 1, 7168)
    return np.ascontiguousarray(out, dtype=np.float32)
